# revision 1
# baseline (speedup 1.0000x reference)
"""Trainium2 Bass kernel for the ANI (anisotropy) L1 loss - final version.

Math (per voxel, 3x3 symmetric tensor, channels xx,xy,xz,yy,yz,zz):
  y_c = gt_std[c]*x_c + gt_mean[c]
  A = [[y0,y1,y2],[y1,y3,y4],[y2,y4,y5]];  q = tr(A)/3;  C = A - q I
  p2 = ||C||_F^2; p = sqrt(p2/6); det = det(C); r = det/(2 p^3)
  phi = arccos(r)/3
  ani_in  = 3 p cos(phi)        ani_tg = q - p cos(phi)
  loss = sum(|ani_in - ani_tg| * mask) / max(sum(mask), 1)

Device identities (ACT tables lack arccos/cos/rsqrt):
  cos(arccos(r)/3) = sin(pi/3 + arctan(w)/3),  w = r/sqrt(1-r^2)
  w = sqrt(6.75) * det / sqrt(gc),  gc = max(e^3 - 6.75 det^2, GMIN)
  where e = p2/2 (the traceless identity sq00+sq11+sq22 = 2(sq00+sq11+n00*n11)
  removes one square; the factor 2 is folded into constant scales).
  3p_in = sqrt(3 e + eps);  p_tg = sqrt(e/3 + eps).

Mapping: bf16 mid-chain on DVE (tensor ops never write in place - that would
break the 2x bf16 perf mode), affines/squares/sqrt/arctan/sin/abs on ScalarE
(Square/Abs live in every ACT table set; the two Sqrt ops are emitted
adjacently per chain to minimize table-set switches), one fused custom DVE op
for gc, RECIPROCAL_APPROX_FAST for 1/gc. Masked |diff| and mask count reduce
to [128,1] partials via accum_out.

Sharding: pure data-parallel, spatial axis split 8 ways. Each core emits
[128,2] (masked-|diff| sum, mask count) partials; the host reduces the 8x128
pairs and divides - that is the "all-reduce of (masked-sum, mask-count)".
"""

import numpy as np

import concourse.tile as tile
from concourse import bacc, mybir
from concourse.bass_utils import run_bass_kernel_spmd

F32 = mybir.dt.float32
BF16 = mybir.dt.bfloat16
I32 = mybir.dt.int32
ALU = mybir.AluOpType
AF = mybir.ActivationFunctionType

N_CORES = 8
B, C = 4, 6
HWD = 96 * 96 * 96
SH = HWD // N_CORES         # spatial elems per core per (b, c)
BSH = B * SH                # 442368 voxels per core
P = 128
FREE = BSH // P             # 3456
NT = 1728                   # free elems per chunk (2 chunks)
XBUF = 3

SQRT675 = float(np.sqrt(6.75))
GMIN = 1e-30
PEPS = 1e-25
PI3 = float(np.pi / 3.0)

_CACHE = {}
_GCLAMP = None
_SQSUM = None


def _register_sqsum():
    """Fused custom DVE op a = sq(in0) + sq(in1) (2x bf16 via perf_en)."""
    global _SQSUM
    if _SQSUM is not None:
        return _SQSUM
    import concourse.dve_ops as dve_ops
    from concourse.dve_ops import DveOp
    from concourse.dve_spec import Spec, Src0, Src1, sq, lower, _has_src1
    from concourse.dve_uop import DveOpSpec

    name = "ANI_SQSUM"
    if name in dve_ops._SUB_OPCODE_FOR_NAME:
        _SQSUM = next(o for o in dve_ops.OPS if o.name == name)
        return _SQSUM
    body = sq(Src0) + sq(Src1)

    def ref(in0, in1, c0, c1, c2):
        a = in0.astype(np.float32)
        b = in1.astype(np.float32)
        return a * a + b * b

    spec = Spec(body=body, reference=ref)
    row = dve_ops._CUSTOM_DVE_ROW_BASE + len(dve_ops.OPS)
    tmp = DveOpSpec(name=name, opcode=row, uops=lower(spec, ver="v3"),
                    rd1_en=_has_src1(spec))
    op = DveOp(name, spec, subdim=False, uops_sha={"v3": tmp.sha("v3")},
               perf_en={"v3": True})
    dve_ops.OPS.append(op)
    dve_ops.CUSTOM_DVE_SPECS[name] = spec
    dve_ops._SUB_OPCODE_FOR_NAME[name] = row
    _SQSUM = op
    return op


def _register_gclamp():
    """Register the fused custom DVE op gc = max(in0^3 - imm2*in1^2, s0)."""
    global _GCLAMP
    if _GCLAMP is not None:
        return _GCLAMP
    import concourse.dve_ops as dve_ops
    from concourse.dve_ops import DveOp
    from concourse.dve_spec import Spec, Src0, Src1, C0, C2, maxx, sq, lower, _has_src1
    from concourse.dve_uop import DveOpSpec

    name = "ANI_GCLAMP"
    if name in dve_ops._SUB_OPCODE_FOR_NAME:
        # already registered (possibly by another module instance); reusing
        # the existing row keeps name->row consistent with any compiled NEFF
        _GCLAMP = next(o for o in dve_ops.OPS if o.name == name)
        return _GCLAMP
    body = maxx((sq(Src0) * Src0) - (sq(Src1) * C2), C0)

    def ref(in0, in1, c0, c1, c2):
        x = in0.astype(np.float32)
        d = in1.astype(np.float32)
        return np.maximum(x * x * x - d * d * c2, c0)

    spec = Spec(body=body, reference=ref)
    row = dve_ops._CUSTOM_DVE_ROW_BASE + len(dve_ops.OPS)
    tmp = DveOpSpec(name=name, opcode=row, uops=lower(spec, ver="v3"),
                    rd1_en=_has_src1(spec))
    op = DveOp(name, spec, subdim=False, uops_sha={"v3": tmp.sha("v3")})
    dve_ops.OPS.append(op)
    dve_ops.CUSTOM_DVE_SPECS[name] = spec
    dve_ops._SUB_OPCODE_FOR_NAME[name] = row
    _GCLAMP = op
    return op


def _build(reps: int = 1):
    NCH = FREE // NT
    gclamp = _register_gclamp()
    sqsum = _register_sqsum()
    nc = bacc.Bacc("TRN2", target_bir_lowering=False, debug=False,
                   num_devices=N_CORES)
    x_in = nc.dram_tensor("input_data", [C, BSH], F32, kind="ExternalInput")
    t_in = nc.dram_tensor("target", [C, BSH], F32, kind="ExternalInput")
    m_in = nc.dram_tensor("mask", [BSH], BF16, kind="ExternalInput")
    sc_in = nc.dram_tensor("scal", [P, 16], F32, kind="ExternalInput")
    out = nc.dram_tensor("out", [P, 1], F32, kind="ExternalOutput")

    with tile.TileContext(nc) as tc:
        with (
            tc.tile_pool(name="const", bufs=1) as cpool,
            tc.tile_pool(name="xio", bufs=XBUF) as xpool,
            tc.tile_pool(name="mio", bufs=1) as mpool,
            tc.tile_pool(name="tmp", bufs=1) as tpool,
            tc.tile_pool(name="acc", bufs=1) as apool,
            tc.tile_pool(name="part", bufs=2) as ppool,
        ):
            scal = cpool.tile([P, 16], F32, tag="scal")
            nc.sync.dma_start(scal[:], sc_in[:])
            lacc = apool.tile([P, 1], F32, tag="lacc")
            nc.vector.memset(lacc[:], 0.0)

            def s_ap(c):
                return scal[:, c:c + 1]

            def mu_ap(c):
                return scal[:, 6 + c:7 + c]

            peps_ap = scal[:, 12:13]
            pi3_ap = scal[:, 13:14]

            _cnt = [0]

            def bt(tag):
                _cnt[0] += 1
                return tpool.tile([P, NT], BF16, tag=tag,
                                  name=f"b{tag}_{_cnt[0]}")

            def ft(tag):
                _cnt[0] += 1
                return tpool.tile([P, NT], F32, tag=tag,
                                  name=f"f{tag}_{_cnt[0]}")

            def chain(src, off, nm, p_scale):
                """One tensor's full chain. Tag namespace `nm` keeps the two
                tensors' chains independent so the scheduler interleaves them.
                Pool slots are recycled across stages (comments note the dead
                tile being replaced); no DVE op ever writes its own input.
                Returns (p, cs, tr2): p = sqrt(p_scale*e+eps), cs = cos term.
                """
                y = []
                for c in range(C):
                    _cnt[0] += 1
                    xt = xpool.tile([P, NT], F32, tag="x",
                                    name=f"x{nm}{c}_{_cnt[0]}")
                    nc.sync.dma_start(
                        xt[:],
                        src[c].rearrange("(p f) -> p f", p=P)[:, off:off + NT])
                    yt = bt(f"{nm}y{c}")
                    nc.scalar.activation(yt[:], xt[:], AF.Identity,
                                         bias=mu_ap(c), scale=s_ap(c))
                    y.append(yt)

                tr = bt(f"{nm}sa")
                nc.vector.tensor_tensor(tr[:], y[0][:], y[3][:], ALU.add)
                tr2 = bt(f"{nm}tr2")
                nc.vector.tensor_tensor(tr2[:], tr[:], y[5][:], ALU.add)
                q = bt(f"{nm}q")
                nc.vector.tensor_scalar(q[:], tr2[:], 1.0 / 3.0, None, ALU.mult)
                n00 = bt(f"{nm}n00")     # negated deviator diag: n = q - y
                nc.vector.tensor_tensor(n00[:], q[:], y[0][:], ALU.subtract)
                n11 = bt(f"{nm}n11")
                nc.vector.tensor_tensor(n11[:], q[:], y[3][:], ALU.subtract)
                n22 = bt(f"{nm}n22")
                nc.vector.tensor_tensor(n22[:], q[:], y[5][:], ALU.subtract)

                o1 = bt(f"{nm}o1")
                nc.scalar.activation(o1[:], y[1][:], AF.Square)
                o2 = bt(f"{nm}o2")
                nc.scalar.activation(o2[:], y[2][:], AF.Square)
                o3 = bt(f"{nm}o3")
                nc.scalar.activation(o3[:], y[4][:], AF.Square)

                pm = bt(f"{nm}q")        # q dead
                nc.vector.tensor_tensor(pm[:], n00[:], n11[:], ALU.mult)

                # e = p2/2 = (sq(n00)+sq(n11)+pm) + (o1+o2+o3), clamped >= 0
                a1 = bt(f"{nm}sa")       # tr dead
                nc.vector._custom_dve(sqsum, out=a1[:], in0=n00[:], in1=n11[:])
                a2 = bt(f"{nm}y5")       # y5 dead
                nc.vector.tensor_tensor(a2[:], a1[:], pm[:], ALU.add)
                b1 = bt(f"{nm}b1")
                nc.vector.tensor_tensor(b1[:], o1[:], o2[:], ALU.add)
                b2 = bt(f"{nm}sa")       # a1 dead
                nc.vector.tensor_tensor(b2[:], b1[:], o3[:], ALU.add)
                e0 = bt(f"{nm}p0")
                nc.vector.tensor_tensor(e0[:], a2[:], b2[:], ALU.add)
                e = bt(f"{nm}p2")        # bf16 rounding can leave e0 < 0 in
                nc.vector.tensor_scalar(  # near-isotropic voxels; sqrt needs >=0
                    e[:], e0[:], 0.0, None, ALU.max)

                # det = n22*(o1 - pm) + n00*o3 + n11*o2 + 2*y1*y2*y4
                K = bt(f"{nm}b1")        # b1 dead
                nc.vector.tensor_tensor(K[:], o1[:], pm[:], ALU.subtract)
                T1 = bt(f"{nm}sa")       # b2 dead
                nc.vector.tensor_tensor(T1[:], n22[:], K[:], ALU.mult)
                A_ = bt(f"{nm}y0")       # sq00 dead
                nc.vector.tensor_tensor(A_[:], n00[:], o3[:], ALU.mult)
                B_ = bt(f"{nm}y3")       # sq11 dead
                nc.vector.tensor_tensor(B_[:], n11[:], o2[:], ALU.mult)
                S_ = bt(f"{nm}o2")       # o2 dead
                nc.vector.tensor_tensor(S_[:], A_[:], B_[:], ALU.add)
                D_ = bt(f"{nm}y5")       # a2 dead
                nc.vector.tensor_tensor(D_[:], T1[:], S_[:], ALU.add)
                Y1 = bt(f"{nm}o3")       # o3 dead
                nc.vector.tensor_tensor(Y1[:], y[1][:], y[2][:], ALU.mult)
                Y2 = bt(f"{nm}y1")       # y1 dead
                nc.vector.tensor_tensor(Y2[:], Y1[:], y[4][:], ALU.mult)
                Z_ = bt(f"{nm}y2")       # y2 dead
                nc.vector.tensor_scalar(Z_[:], Y2[:], 2.0, None, ALU.mult)
                det = bt(f"{nm}det")
                nc.vector.tensor_tensor(det[:], D_[:], Z_[:], ALU.add)

                # gc = max(e^3 - 6.75 det^2, GMIN)  (= (p2^3 - 54 det^2)/8)
                gc = ft(f"{nm}gc")
                nc.vector._custom_dve(gclamp, out=gc[:], in0=e[:], in1=det[:],
                                      s0=GMIN, imm2=6.75)
                rec = ft(f"{nm}rec")
                nc.vector.reciprocal_approx_fast(rec[:], gc[:])

                # both Sqrt ops adjacent (one sqrt-table load), then trig set
                p = bt(f"{nm}p0")        # e0 dead
                nc.scalar.activation(p[:], e[:], AF.Sqrt,
                                     bias=peps_ap, scale=p_scale)
                rsg = bt(f"{nm}n11")     # n11 dead
                nc.scalar.activation(rsg[:], rec[:], AF.Sqrt)
                wk = bt(f"{nm}n22")      # n22 dead
                nc.vector.tensor_tensor(wk[:], det[:], rsg[:], ALU.mult)
                nc.scalar.activation(wk[:], wk[:], AF.Arctan, scale=SQRT675)
                nc.scalar.activation(wk[:], wk[:], AF.Sin,
                                     bias=pi3_ap, scale=1.0 / 3.0)   # cs
                return p, wk, tr2

            for _ in range(reps):
                for ch in range(NCH):
                    off = ch * NT
                    _cnt[0] += 1
                    mf = mpool.tile([P, NT], BF16, tag="mask",
                                    name=f"mask_{_cnt[0]}")
                    nc.sync.dma_start(
                        mf[:],
                        m_in.rearrange("(p f) -> p f", p=P)[:, off:off + NT])

                    p3i, csi, _ = chain(x_in, off, "i", 3.0)       # 3 p_in
                    p1t, cst, tr2t = chain(t_in, off, "t", 1.0 / 3.0)

                    u = bt("u")
                    nc.vector.tensor_tensor(u[:], p3i[:], csi[:], ALU.mult)
                    v = bt("v")
                    nc.vector.tensor_tensor(v[:], p1t[:], cst[:], ALU.mult)
                    w3 = bt("w3")
                    nc.vector.tensor_tensor(w3[:], u[:], v[:], ALU.add)
                    qt = bt("qt")
                    nc.vector.tensor_scalar(qt[:], tr2t[:], 1.0 / 3.0,
                                            None, ALU.mult)
                    nd = bt("nd")        # q_t - (ani_in + p_t cs_t); |.| later
                    nc.vector.tensor_tensor(nd[:], qt[:], w3[:], ALU.subtract)
                    dm = bt("v")         # v dead
                    nc.vector.tensor_tensor(dm[:], nd[:], mf[:], ALU.mult)

                    asum = ppool.tile([P, 1], F32, tag="asum")
                    nc.scalar.activation(dm[:], dm[:], AF.Abs,
                                         accum_out=asum[:])
                    nc.vector.tensor_tensor(lacc[:], lacc[:], asum[:], ALU.add)

            nc.sync.dma_start(out[:], lacc[:])

    nc.compile()
    return nc


def get_module(reps: int = 1):
    if reps not in _CACHE:
        _CACHE[reps] = _build(reps)
    return _CACHE[reps]


def make_in_maps(input_data, target, mask, gt_mean, gt_std):
    """Shard the full inputs 8 ways along the flattened spatial axis; each
    core gets contiguous per-channel planes [C, B*SH]."""
    import ml_dtypes
    xs = np.asarray(input_data, np.float32).reshape(B, C, HWD)
    ts = np.asarray(target, np.float32).reshape(B, C, HWD)
    ms = np.asarray(mask, np.int32).reshape(B, HWD)
    scal = np.zeros((P, 16), np.float32)
    scal[:, 0:6] = np.asarray(gt_std, np.float32).reshape(1, 6)
    scal[:, 6:12] = np.asarray(gt_mean, np.float32).reshape(1, 6)
    scal[:, 12] = PEPS
    scal[:, 13] = PI3
    in_maps = []
    for k in range(N_CORES):
        sl = slice(k * SH, (k + 1) * SH)
        in_maps.append({
            "input_data": np.ascontiguousarray(
                xs[:, :, sl].transpose(1, 0, 2)).reshape(C, BSH),
            "target": np.ascontiguousarray(
                ts[:, :, sl].transpose(1, 0, 2)).reshape(C, BSH),
            "mask": np.ascontiguousarray(ms[:, sl]).reshape(BSH).astype(
                ml_dtypes.bfloat16),
            "scal": scal,
        })
    return in_maps


def kernel(input_data, target, mask, gt_mean, gt_std):
    nc = get_module()
    in_maps = make_in_maps(input_data, target, mask, gt_mean, gt_std)
    n = float(np.asarray(mask, np.int64).sum())   # mask count on host
    r = run_bass_kernel_spmd(nc, in_maps, core_ids=list(range(N_CORES)))
    s = 0.0
    for i in range(N_CORES):
        s += r.results[i]["out"].astype(np.float64).sum()
    return np.float32(s / max(n, 1.0))



# revision 2
# speedup vs baseline: 1.0509x; 1.0509x over previous
"""Trainium2 Bass kernel for the ANI (anisotropy) L1 loss — v3.

Same math as kernel2 (validated in check_math.py):
  nt = n22/2 and dh = (y3-y0)/3 from third-scaled diagonals yhat = y/3:
    e     = 3 nt^2 + 2.25 dh^2 + q1 + q2 + q4      (>= 0 by construction)
    det/2 = nt(q1 - (q2+q4)/2) + (2.25 nt dh^2 - nt^3) + 0.75 dh (q4-q2)
            + y1 y2 y4
    gc = max(e^3 - 27 (det/2)^2, GMIN); w = sqrt(27) (det/2) / sqrt(gc)
    p*cs with p = sqrt(3e + eps), cs = sin(pi/3 + arctan(w)/3)
  Chain scaling mu=1 (input) / mu=1/3 (target) folded into the affine
  consts makes |ani_in - ani_tg| = |(p cs)_x + (p cs)_t - 3 sum(yhat_t)|.
  Mask-free: host rewrites masked voxels of x,t to -mean/std so y ~ 0 there
  and the voxel contributes ~0 (bias ~0.1%, well under tolerance).

The two chains (input/target) are separate instruction streams,
stage-interleaved so each engine's in-order queue always holds the other
chain's independent work. ACT ops are grouped per table set (squares ->
sqrt -> rsqrt -> trig -> abs) per chunk, three table loads per chunk.
Inputs ship as bf16 (halves HBM traffic vs f32).
"""

import numpy as np

import concourse.tile as tile
from concourse import bacc, mybir
from concourse.bass_utils import run_bass_kernel_spmd

F32 = mybir.dt.float32
BF16 = mybir.dt.bfloat16
ALU = mybir.AluOpType
AF = mybir.ActivationFunctionType

N_CORES = 8
B, C = 4, 6
HWD = 96 * 96 * 96
SH = HWD // N_CORES
BSH = B * SH                # 442368 voxels per core
P = 128
FREE = BSH // P             # 3456
NCH = 2
NT = FREE // NCH            # 1728
XBUF = 2

GMIN = 1e-30
PEPS = 1e-25
PI3 = float(np.pi / 3.0)
SQRT27 = float(np.sqrt(27.0))

_CACHE = {}
_OPS = {}

# Pool-engine offload measured slower in practice (GPSIMD software ALU);
# keep every tensor op on DVE and the transcendentals on ACT.
POOL_CH = ()


def _register(name, body_fn, ref):
    import concourse.dve_ops as dve_ops
    from concourse.dve_ops import DveOp
    from concourse.dve_spec import Spec, lower, _has_src1
    from concourse.dve_uop import DveOpSpec

    if name in _OPS:
        return _OPS[name]
    if name in dve_ops._SUB_OPCODE_FOR_NAME:
        op = next(o for o in dve_ops.OPS if o.name == name)
        _OPS[name] = op
        return op
    spec = Spec(body=body_fn(), reference=ref)
    row = dve_ops._CUSTOM_DVE_ROW_BASE + len(dve_ops.OPS)
    tmp = DveOpSpec(name=name, opcode=row, uops=lower(spec, ver="v3"),
                    rd1_en=_has_src1(spec))
    op = DveOp(name, spec, subdim=False, uops_sha={"v3": tmp.sha("v3")})
    dve_ops.OPS.append(op)
    dve_ops.CUSTOM_DVE_SPECS[name] = spec
    dve_ops._SUB_OPCODE_FOR_NAME[name] = row
    _OPS[name] = op
    return op


def _ops():
    from concourse.dve_spec import Src0, Src1, C0, C1, C2, maxx, sq

    e1 = _register(
        "ANI2_E1",
        lambda: (sq(Src0) * C1) + (sq(Src1) * C2),
        lambda i0, i1, c0, c1, c2: i0.astype(np.float32) ** 2 * c1
        + i1.astype(np.float32) ** 2 * c2)
    cp = _register(
        "ANI2_CP",
        lambda: ((sq(Src1) * Src0) * C2) - (sq(Src0) * Src0),
        lambda i0, i1, c0, c1, c2: i1.astype(np.float32) ** 2
        * i0.astype(np.float32) * c2 - i0.astype(np.float32) ** 3)
    gcl = _register(
        "ANI2_GCLAMP",
        lambda: maxx((sq(Src0) * Src0) - (sq(Src1) * C2), C0),
        lambda i0, i1, c0, c1, c2: np.maximum(
            i0.astype(np.float32) ** 3 - i1.astype(np.float32) ** 2 * c2, c0))
    return e1, cp, gcl


def _build(reps: int = 1):
    cu_e1, cu_cp, cu_gcl = _ops()
    nc = bacc.Bacc("TRN2", target_bir_lowering=False, debug=False,
                   num_devices=N_CORES)
    # layout: 12 channel planes [P, FREE] each: x0..x5 then t0..t5
    xt_in = nc.dram_tensor("xt", [P, 12 * FREE], BF16, kind="ExternalInput")
    sc_in = nc.dram_tensor("scal", [P, 26], F32, kind="ExternalInput")
    out = nc.dram_tensor("out", [P, 1], F32, kind="ExternalOutput")

    with tile.TileContext(nc) as tc:
        with (
            tc.tile_pool(name="const", bufs=1) as cpool,
            tc.tile_pool(name="xio", bufs=XBUF) as xpool,
            tc.tile_pool(name="tmp", bufs=1) as tpool,
            tc.tile_pool(name="acc", bufs=1) as apool,
            tc.tile_pool(name="part", bufs=2) as ppool,
        ):
            scal = cpool.tile([P, 26], F32, tag="scal")
            nc.sync.dma_start(scal[:], sc_in[:])
            lacc = apool.tile([P, 1], F32, tag="lacc")
            nc.vector.memset(lacc[:], 0.0)

            def sc_ap(chain, c):
                return scal[:, 12 * chain + c:12 * chain + c + 1]

            def sh_ap(chain, c):
                return scal[:, 12 * chain + 6 + c:12 * chain + 7 + c]

            peps_ap = scal[:, 24:25]
            pi3_ap = scal[:, 25:26]

            _cnt = [0]

            def nt(pref, tag):
                _cnt[0] += 1
                return tpool.tile([P, NT], BF16, tag=f"{pref}{tag}",
                                  name=f"{pref}{tag}_{_cnt[0]}")

            for _ in range(reps):
                for ch in range(NCH):
                    Y = [[None] * C, [None] * C]
                    # ---- DMA + affines (DVE: 0,3,5; Pool: 1,2,4) ----
                    for cn in range(2):          # 0 = input, 1 = target
                        pf = "i" if cn == 0 else "t"
                        for c in range(C):
                            _cnt[0] += 1
                            xin = xpool.tile([P, NT], BF16, tag=f"x{pf}{c}",
                                             name=f"x{pf}{c}_{_cnt[0]}")
                            off = (cn * C + c) * FREE + ch * NT
                            nc.sync.dma_start(xin[:],
                                              xt_in[:, off:off + NT])
                            y = nt(pf, f"A{c}")
                            eng = nc.gpsimd if c in POOL_CH else nc.vector
                            eng.tensor_scalar(y[:], xin[:], sc_ap(cn, c),
                                              sh_ap(cn, c), ALU.mult, ALU.add)
                            Y[cn][c] = y

                    # ---- stage 1: deviator (DVE) ----
                    S = [{}, {}]
                    for cn in range(2):
                        pf = "i" if cn == 0 else "t"
                        y0, y1, y2, y3, y4, y5 = Y[cn]
                        v = S[cn]
                        v["t1"] = nt(pf, "B0")
                        nc.vector.tensor_tensor(v["t1"][:], y0[:], y3[:],
                                                ALU.add)
                        if cn == 1:
                            v["sq"] = nt(pf, "S1")
                            nc.vector.tensor_tensor(v["sq"][:], v["t1"][:],
                                                    y5[:], ALU.add)
                        v["h"] = nt(pf, "B1")
                        nc.vector.tensor_scalar(v["h"][:], v["t1"][:], 0.5,
                                                None, ALU.mult)
                        v["dh"] = nt(pf, "B2")
                        nc.vector.tensor_tensor(v["dh"][:], y3[:], y0[:],
                                                ALU.subtract)
                        v["nt"] = nt(pf, "A0")       # y0 dead (t1, dh)
                        nc.vector.tensor_tensor(v["nt"][:], v["h"][:], y5[:],
                                                ALU.subtract)

                    # ---- stage 2: ACT squares (any table set) ----
                    for cn in range(2):
                        pf = "i" if cn == 0 else "t"
                        v = S[cn]
                        y = Y[cn]
                        v["q1"] = nt(pf, "A3")       # y3 dead (t1, dh)
                        nc.scalar.activation(v["q1"][:], y[1][:], AF.Square)
                        v["q2"] = nt(pf, "B1")       # h dead (nt)
                        nc.scalar.activation(v["q2"][:], y[2][:], AF.Square)
                        v["q4"] = nt(pf, "B3")
                        nc.scalar.activation(v["q4"][:], y[4][:], AF.Square)

                    # ---- stage 3: products + e (DVE) ----
                    for cn in range(2):
                        pf = "i" if cn == 0 else "t"
                        v = S[cn]
                        y = Y[cn]
                        v["Y1"] = nt(pf, "B4")
                        nc.vector.tensor_tensor(v["Y1"][:], y[1][:], y[2][:],
                                                ALU.mult)
                        v["Y2"] = nt(pf, "A1")       # y1 dead (q1, Y1)
                        nc.vector.tensor_tensor(v["Y2"][:], v["Y1"][:],
                                                y[4][:], ALU.mult)
                        v["E1"] = nt(pf, "A2")       # y2 dead (q2, Y1)
                        nc.vector._custom_dve(cu_e1, out=v["E1"][:],
                                              in0=v["nt"][:], in1=v["dh"][:],
                                              s1=3.0, imm2=2.25)
                        v["CP"] = nt(pf, "A4")       # y4 dead (q4, Y2)
                        nc.vector._custom_dve(cu_cp, out=v["CP"][:],
                                              in0=v["nt"][:], in1=v["dh"][:],
                                              imm2=2.25)
                        v["Sp"] = nt(pf, "A5")       # y5 dead (sq, nt)
                        nc.vector.tensor_tensor(v["Sp"][:], v["q2"][:],
                                                v["q4"][:], ALU.add)
                        v["Sm"] = nt(pf, "B5")
                        nc.vector.tensor_tensor(v["Sm"][:], v["q4"][:],
                                                v["q2"][:], ALU.subtract)
                        v["e1"] = nt(pf, "B1")       # q2 dead (Sp, Sm)
                        nc.vector.tensor_tensor(v["e1"][:], v["E1"][:],
                                                v["Sp"][:], ALU.add)
                        v["e"] = nt(pf, "B4")
                        nc.vector.tensor_tensor(v["e"][:], v["e1"][:],
                                                v["q1"][:], ALU.add)

                    # ---- stage 4: ACT sqrt set ----
                    for cn in range(2):
                        pf = "i" if cn == 0 else "t"
                        v = S[cn]
                        v["p"] = nt(pf, "B1")
                        nc.scalar.activation(v["p"][:], v["e"][:], AF.Sqrt,
                                             bias=peps_ap, scale=3.0)

                    # ---- stage 5: det + gc (DVE) ----
                    for cn in range(2):
                        pf = "i" if cn == 0 else "t"
                        v = S[cn]
                        v["Sh"] = nt(pf, "B3")
                        nc.vector.tensor_scalar(v["Sh"][:], v["Sp"][:], 0.5,
                                                None, ALU.mult)
                        v["G1"] = nt(pf, "A5")       # Sp dead (e1, Sh)
                        nc.vector.tensor_tensor(v["G1"][:], v["q1"][:],
                                                v["Sh"][:], ALU.subtract)
                        v["M2"] = nt(pf, "A3")       # q1 dead (e, G1)
                        nc.vector.tensor_tensor(v["M2"][:], v["nt"][:],
                                                v["G1"][:], ALU.mult)
                        v["V"] = nt(pf, "B3")       # q4 dead (Sp, Sm)
                        nc.vector.tensor_tensor(v["V"][:], v["dh"][:],
                                                v["Sm"][:], ALU.mult)
                        v["Vq"] = nt(pf, "B5")       # Sm dead (V)
                        nc.vector.tensor_scalar(v["Vq"][:], v["V"][:], 0.75,
                                                None, ALU.mult)
                        v["D1"] = nt(pf, "A0")       # nt dead (E1, CP, M2)
                        nc.vector.tensor_tensor(v["D1"][:], v["CP"][:],
                                                v["M2"][:], ALU.add)
                        v["D2"] = nt(pf, "B2")       # dh dead (E1, CP, V)
                        nc.vector.tensor_tensor(v["D2"][:], v["Vq"][:],
                                                v["Y2"][:], ALU.add)
                        v["dh2"] = nt(pf, "A4")
                        nc.vector.tensor_tensor(v["dh2"][:], v["D1"][:],
                                                v["D2"][:], ALU.add)
                        _cnt[0] += 1
                        v["gc"] = tpool.tile([P, NT], F32, tag=f"{pf}GF",
                                             name=f"{pf}gc_{_cnt[0]}")
                        nc.vector._custom_dve(cu_gcl, out=v["gc"][:],
                                              in0=v["e"][:], in1=v["dh2"][:],
                                              s0=GMIN, imm2=27.0)

                    # ---- stage 6: ACT rsqrt set ----
                    for cn in range(2):
                        pf = "i" if cn == 0 else "t"
                        v = S[cn]
                        v["rsg"] = nt(pf, "A1")      # Y2 dead (D2)
                        nc.scalar.activation(v["rsg"][:], v["gc"][:],
                                             AF.Abs_reciprocal_sqrt)

                    # ---- stage 7: wk (DVE) ----
                    for cn in range(2):
                        pf = "i" if cn == 0 else "t"
                        v = S[cn]
                        v["wk"] = nt(pf, "A2")       # E1 dead (e1)
                        nc.vector.tensor_tensor(v["wk"][:], v["dh2"][:],
                                                v["rsg"][:], ALU.mult)

                    # ---- stage 8: ACT trig set ----
                    for cn in range(2):
                        pf = "i" if cn == 0 else "t"
                        v = S[cn]
                        v["at"] = nt(pf, "B3")      # G1 dead (M2)
                        nc.scalar.activation(v["at"][:], v["wk"][:],
                                             AF.Arctan, scale=SQRT27)
                        v["cs"] = nt(pf, "B5")      # Vq dead (D2)
                        nc.scalar.activation(v["cs"][:], v["at"][:], AF.Sin,
                                             bias=pi3_ap, scale=1.0 / 3.0)

                    # ---- stage 9: combine (DVE) + abs accum (ACT) ----
                    umi = nt("i", "B4")
                    nc.vector.tensor_tensor(umi[:], S[0]["p"][:],
                                            S[0]["cs"][:], ALU.mult)
                    umt = nt("t", "B4")
                    nc.vector.tensor_tensor(umt[:], S[1]["p"][:],
                                            S[1]["cs"][:], ALU.mult)
                    nd1 = nt("c", "S2")
                    nc.vector.tensor_tensor(nd1[:], umi[:], umt[:], ALU.add)
                    sq3 = nt("c", "S3")
                    nc.vector.tensor_scalar(sq3[:], S[1]["sq"][:], 3.0,
                                            None, ALU.mult)
                    nd = nt("c", "S4")
                    nc.vector.tensor_tensor(nd[:], nd1[:], sq3[:],
                                            ALU.subtract)
                    asum = ppool.tile([P, 1], F32, tag="asum")
                    nd2 = nt("c", "S2")              # nd1 dead
                    nc.scalar.activation(nd2[:], nd[:], AF.Abs,
                                         accum_out=asum[:])
                    nc.vector.tensor_tensor(lacc[:], lacc[:], asum[:],
                                            ALU.add)

            nc.sync.dma_start(out[:], lacc[:])

    nc.compile()
    return nc


def get_module(reps: int = 1):
    if reps not in _CACHE:
        _CACHE[reps] = _build(reps)
    return _CACHE[reps]


def make_in_maps(input_data, target, mask, gt_mean, gt_std):
    import ml_dtypes

    s = np.asarray(gt_std, np.float32).reshape(C)
    m = np.asarray(gt_mean, np.float32).reshape(C)
    xs = np.asarray(input_data, np.float32).reshape(B, C, HWD).copy()
    ts = np.asarray(target, np.float32).reshape(B, C, HWD).copy()
    msk = np.asarray(mask, np.int32).reshape(B, 1, HWD)

    # mask-free trick: masked voxels -> x = t = -m/s so y = s*x+m ~ 0
    xm = (-m / s).astype(np.float32).reshape(1, C, 1)
    dead = np.broadcast_to(msk == 0, xs.shape)
    xs[dead] = np.broadcast_to(xm, xs.shape)[dead]
    ts[dead] = np.broadcast_to(xm, ts.shape)[dead]

    # affine consts: input chain mu=1, target chain mu=1/3; diag extra 1/3
    DIAG = (0, 3, 5)
    scal = np.zeros((P, 26), np.float32)
    for c in range(C):
        d3 = 3.0 if c in DIAG else 1.0
        scal[:, c] = s[c] / d3
        scal[:, 6 + c] = m[c] / d3
        scal[:, 12 + c] = s[c] / (3.0 * d3)
        scal[:, 18 + c] = m[c] / (3.0 * d3)
    scal[:, 24] = PEPS
    scal[:, 25] = PI3

    xs16 = xs.astype(ml_dtypes.bfloat16)
    ts16 = ts.astype(ml_dtypes.bfloat16)
    in_maps = []
    for k in range(N_CORES):
        sl = slice(k * SH, (k + 1) * SH)
        xc = np.ascontiguousarray(
            xs16[:, :, sl].transpose(1, 0, 2)).reshape(C, P, FREE)
        tc_ = np.ascontiguousarray(
            ts16[:, :, sl].transpose(1, 0, 2)).reshape(C, P, FREE)
        xt = np.concatenate([xc, tc_], axis=0)       # [12, P, FREE]
        in_maps.append({
            "xt": np.ascontiguousarray(
                xt.transpose(1, 0, 2)).reshape(P, 12 * FREE),
            "scal": scal,
        })
    return in_maps


def kernel(input_data, target, mask, gt_mean, gt_std):
    nc = get_module()
    in_maps = make_in_maps(input_data, target, mask, gt_mean, gt_std)
    n = float(np.asarray(mask, np.int64).sum())
    r = run_bass_kernel_spmd(nc, in_maps, core_ids=list(range(N_CORES)))
    total = 0.0
    for i in range(N_CORES):
        total += r.results[i]["out"].astype(np.float64).sum()
    return np.float32(total / max(n, 1.0))
